# revision 1
# baseline (speedup 1.0000x reference)
"""Trainium2 Bass kernel for nn_EncoderLayer (D=512, H=8, DK=64, DF=2048, B=2, S=2048).

Strategy (8 NeuronCores):
  - Attention is head-parallel: core c computes head c for both batches.
    All on-chip attention work happens in transposed layout (features on
    partitions, tokens on the free dim) so no large transposes are needed:
    the host passes x pre-transposed (xT[b] = x[b].T).
  - The softmax denominator is fused into the attention-output matmul by
    augmenting V with a ones column (out row 64 = sum of exp scores).
    Softmax max-subtraction is skipped: scores*0.125 is O(1) here, exp is
    safely in range, and exp(s)/sum(exp(s)) is exact regardless.
  - One AllToAll (1 MB/core) redistributes per-head attention outputs to
    per-token shards; each core then does Wo projection + LN1 + FFN + LN2
    for its 512 tokens and returns its output shard.
"""

import numpy as np

import concourse.bass as bass
import concourse.tile as tile
from concourse import mybir
from concourse.bass_utils import run_bass_kernel_spmd
from concourse.masks import make_identity
from concourse.vector_clock import ScopedClock

F32 = mybir.dt.float32
F32R = mybir.dt.float32r
BF16 = mybir.dt.bfloat16
AF = mybir.ActivationFunctionType
ALU = mybir.AluOpType

B, S, D, H, DK, DF = 2, 2048, 512, 8, 64, 2048
N_CORES = 8
NSH = (B * S) // N_CORES  # tokens per core shard = 512
EPS = 1e-5

# ---------------------------------------------------------------------------
# Workaround: this walrus build rejects >1 sem wait on CTRL-type (drain)
# instructions. Split the TileContext tail-drain waits onto dedicated
# single-wait nops; the all-engine barrier right after keeps semantics.


def _split_excess_waits(nc, max_waits=1):
    """Hoist sem waits beyond `max_waits` onto dedicated single-wait nops
    inserted just before the instruction on the same engine queue."""
    for fn in nc.m.functions:
        for bb in fn.blocks:
            new_list = []
            for inst in bb.instructions:
                si = inst.sync_info
                waits = list(si.on_wait) if si is not None and si.on_wait else []
                if len(waits) > max_waits:
                    keep = waits[:max_waits]
                    extra = waits[max_waits:]
                    si.on_wait = keep
                    for w in extra:
                        nop = mybir.InstNoOp(name=f"I-waitnop-{nc.next_id()}")
                        nop.engine = inst.engine
                        nop.sync_info = mybir.SyncInfo(on_wait=[w], on_update=[])
                        new_list.append(nop)
                new_list.append(inst)
            bb.instructions = new_list


def _bcast_ap(handle, parts, n):
    """AP reading a 1-D DRAM tensor broadcast across `parts` partitions."""
    a = handle[:]
    return bass.AP(tensor=a.tensor, offset=a.offset, ap=[[0, parts], [1, n]])


def build_nc():
    nc = bass.Bass(target_bir_lowering=False)

    # ---- kernel I/O (per core) ----
    xT = nc.dram_tensor("xT", [B, D, S], F32R, kind="ExternalInput")
    xsb = nc.dram_tensor("xsb", [NSH, D], F32, kind="ExternalInput")  # x_shard + bo
    wq = nc.dram_tensor("wq", [D, DK], F32R, kind="ExternalInput")
    wk = nc.dram_tensor("wk", [D, DK], F32R, kind="ExternalInput")
    wv = nc.dram_tensor("wv", [D, DK], F32R, kind="ExternalInput")
    bq = nc.dram_tensor("bq", [DK, 1], F32, kind="ExternalInput")
    bk = nc.dram_tensor("bk", [DK, 1], F32, kind="ExternalInput")
    bv = nc.dram_tensor("bv", [DK], F32, kind="ExternalInput")
    wo = nc.dram_tensor("wo", [H * DK, D], F32R, kind="ExternalInput")
    w1 = nc.dram_tensor("w1", [D, DF], F32R, kind="ExternalInput")
    b1c = nc.dram_tensor("b1c", [128, DF // 128], F32, kind="ExternalInput")
    w2 = nc.dram_tensor("w2", [DF, D], F32R, kind="ExternalInput")
    b2 = nc.dram_tensor("b2", [D], F32R, kind="ExternalInput")
    g1 = nc.dram_tensor("g1", [D], F32, kind="ExternalInput")
    be1 = nc.dram_tensor("be1", [D], F32, kind="ExternalInput")
    g2 = nc.dram_tensor("g2", [D], F32, kind="ExternalInput")
    be2 = nc.dram_tensor("be2", [D], F32, kind="ExternalInput")
    out_shard = nc.dram_tensor("out_shard", [NSH, D], F32, kind="ExternalOutput")

    with tile.TileContext(nc) as tc:
        with (
            tc.tile_pool(name="consts", bufs=1) as consts,
            tc.tile_pool(name="xt", bufs=4) as xt_pool,
            tc.tile_pool(name="qk", bufs=2) as qk_pool,
            tc.tile_pool(name="vaug", bufs=2) as v_pool,
            tc.tile_pool(name="expt", bufs=3) as exp_pool,
            tc.tile_pool(name="otn", bufs=3) as ot_pool,
            tc.tile_pool(name="wff", bufs=3) as wff_pool,
            tc.tile_pool(name="f1p", bufs=3) as f1_pool,
            tc.tile_pool(name="hh", bufs=2) as h_pool,
            tc.tile_pool(name="tmps", bufs=2) as tmp_pool,
            tc.tile_pool(name="small", bufs=2) as small,
            tc.tile_pool(name="psc", bufs=2, space="PSUM") as psc,
            tc.tile_pool(name="pacc", bufs=4, space="PSUM") as pacc,
            tc.tile_pool(name="dram", bufs=1, space="DRAM") as dram,
        ):
            # ---- first: start streaming x (b=0) and the QKV weights so the
            # PE can begin as early as possible; other consts follow.
            xt_first = []
            for d in range(4):
                t_ = xt_pool.tile([128, S], F32R, tag="xt", name=f"xt0_{d}")
                nc.sync.dma_start(out=t_, in_=xT[0, 128 * d : 128 * (d + 1), :])
                xt_first.append(t_)
            wq_sb = consts.tile([128, 4 * DK], F32R, tag="wq_sb")
            wk_sb = consts.tile([128, 4 * DK], F32R, tag="wk_sb")
            wv_sb = consts.tile([128, 4 * DK], F32R, tag="wv_sb")
            for w_sb, w_h in ((wq_sb, wq), (wk_sb, wk), (wv_sb, wv)):
                nc.sync.dma_start(
                    out=w_sb[:].rearrange("p (d k) -> p d k", k=DK),
                    in_=w_h[:, :].rearrange("(d p) k -> p d k", p=128),
                )
            bq_sb = consts.tile([DK, 1], F32, tag="bq_sb")
            bk_sb = consts.tile([DK, 1], F32, tag="bk_sb")
            nc.sync.dma_start(out=bq_sb, in_=bq[:, :])
            nc.sync.dma_start(out=bk_sb, in_=bk[:, :])

            ident = consts.tile([128, 128], F32)
            make_identity(nc, ident)
            eps_t = consts.tile([128, 1], F32)
            nc.vector.memset(eps_t, EPS)
            ones1 = consts.tile([1, DK], F32R)
            nc.vector.memset(ones1[:].bitcast(F32), 1.0)
            bv_bc = consts.tile([128, DK], F32)
            nc.gpsimd.dma_start(out=bv_bc, in_=_bcast_ap(bv, 128, DK))
            bv8 = consts.tile([128, 8 * DK], F32)
            for i in range(8):
                nc.vector.tensor_copy(bv8[:, i * DK : (i + 1) * DK], bv_bc)

            wo_sb = consts.tile([128, 4 * D], F32R, tag="wo_sb")
            nc.sync.dma_start(
                out=wo_sb[:].rearrange("p (c d) -> p c d", d=D),
                in_=wo[:, :].rearrange("(c p) d -> p c d", p=128),
            )
            b1_sb = consts.tile([128, DF // 128], F32, tag="b1_sb")
            nc.sync.dma_start(out=b1_sb, in_=b1c[:, :])

            b2r = consts.tile([1, D], F32R, tag="b2r")
            nc.sync.dma_start(out=b2r, in_=b2[:].rearrange("(o d) -> o d", o=1))
            ones128 = consts.tile([1, 128], F32R)
            nc.vector.memset(ones128[:].bitcast(F32), 1.0)
            g1_t = consts.tile([128, D], F32, tag="g1_t")
            be1_t = consts.tile([128, D], F32, tag="be1_t")
            g2_t = consts.tile([128, D], F32, tag="g2_t")
            be2_t = consts.tile([128, D], F32, tag="be2_t")
            for t_sb, h_d in ((g1_t, g1), (be1_t, be1), (g2_t, g2), (be2_t, be2)):
                nc.gpsimd.dma_start(out=t_sb, in_=_bcast_ap(h_d, 128, D))

            xsbo = []
            for i in range(4):
                xt_ = consts.tile([128, D], F32, tag=f"xsbo{i}")
                nc.sync.dma_start(out=xt_, in_=xsb[128 * i : 128 * (i + 1), :])
                xsbo.append(xt_)

            # Per-batch exchange buffers: block j (16 rows x 1024) carries
            # oT[:, 256j:256(j+1)] for that batch, bound for core j.
            send_h = [dram.tile([128, 1024], F32R, name=f"send{b}") for b in range(B)]
            recv_h = [dram.tile([128, 1024], F32R, name=f"recv{b}") for b in range(B)]

            last_send = [None, None]

            # =========== per-batch: QKV + attention ===========
            xt_next = None
            for b in range(B):
                xt = xt_first if b == 0 else xt_next

                # qT/kT [128, 2048]: rows 0-63 = projection, rows 64-127
                # zeroed so the scores matmul can run K=128 (fp32r K=64 MMs
                # do not register as PE activity for the HAM clock-gate and
                # run at 1.2 GHz; K=128 keeps the PE warm at 2.4 GHz).
                qT = qk_pool.tile([128, S], F32R, tag="qT")
                kT = qk_pool.tile([128, S], F32R, tag="kT")
                nc.vector.memset(qT[64:128, :].bitcast(F32), 0.0)
                nc.vector.memset(kT[64:128, :].bitcast(F32), 0.0)
                for dst, w_sb, b_sb in ((qT, wq_sb, bq_sb), (kT, wk_sb, bk_sb)):
                    for s4 in range(4):
                        ps = psc.tile([DK, 512], F32, tag="sc")
                        for d in range(4):
                            nc.tensor.matmul(
                                ps,
                                lhsT=w_sb[:, DK * d : DK * (d + 1)],
                                rhs=xt[d][:, 512 * s4 : 512 * (s4 + 1)],
                                start=(d == 0),
                                stop=(d == 3),
                            )
                        nc.vector.tensor_scalar_add(
                            dst[0:DK, 512 * s4 : 512 * (s4 + 1)], ps, b_sb
                        )

                # v_aug [128(t), 16*65]: per t-chunk 64 v columns + a ones column
                v_aug = v_pool.tile([128, 16 * (DK + 1)], BF16, tag="vaug")
                nc.vector.memset(v_aug, 1.0)
                v_view = v_aug[:].rearrange("p (t c) -> p t c", c=DK + 1)
                for half in range(2):
                    psv = psc.tile([128, 512], F32, tag="sc")
                    for t8 in range(8):
                        t = 8 * half + t8
                        for d in range(4):
                            nc.tensor.matmul(
                                psv[:, DK * t8 : DK * (t8 + 1)],
                                lhsT=xt[d][:, 128 * t : 128 * (t + 1)],
                                rhs=wv_sb[:, DK * d : DK * (d + 1)],
                                start=(d == 0),
                                stop=(d == 3),
                            )
                    nc.vector.tensor_tensor(
                        out=v_view[:, 8 * half : 8 * half + 8, 0:DK],
                        in0=psv[:].rearrange("p (t c) -> p t c", c=DK),
                        in1=bv8[:].rearrange("p (t c) -> p t c", c=DK),
                        op=ALU.add,
                    )

                # ---- attention: scoresT -> exp -> o accumulation ----
                # Software-pipelined: the o-matmuls for chunk t-1 are emitted
                # after the exp of chunk t, so the PE streams scores(t) and
                # o(t-1) back-to-back while ACT computes exp(t).
                o_ps = [pacc.tile([DK + 1, 512], F32, tag="acc", name=f"ops{b}_{i}") for i in range(4)]
                et_prev = None

                def emit_o(t, et_t):
                    for i in range(4):
                        nc.tensor.matmul(
                            o_ps[i],
                            lhsT=v_view[:, t, :],
                            rhs=et_t[:, 512 * i : 512 * (i + 1)],
                            start=(t == 0),
                            stop=(t == 15),
                        )

                for t in range(16):
                    et = exp_pool.tile([128, S], BF16, tag="et")
                    for half in range(2):
                        ps_sc = psc.tile([128, 1024], F32, tag="sc")
                        for sq in range(2):
                            s4 = 2 * half + sq
                            nc.tensor.matmul(
                                ps_sc[:, 512 * sq : 512 * (sq + 1)],
                                lhsT=kT[:, 128 * t : 128 * (t + 1)],
                                rhs=qT[:, 512 * s4 : 512 * (s4 + 1)],
                                start=True,
                                stop=True,
                            )
                        nc.scalar.activation(
                            out=et[:, 1024 * half : 1024 * (half + 1)],
                            in_=ps_sc,
                            func=AF.Exp,
                            bias=0.0,
                            scale=0.125,
                        )
                    if et_prev is not None:
                        emit_o(t - 1, et_prev)
                    et_prev = et
                emit_o(15, et_prev)

                if b == 0:
                    # emit b1's x loads before the send DMAs so they are not
                    # stuck behind collective-gated descriptors in the queues
                    xt_next = []
                    for d in range(4):
                        t_ = xt_pool.tile([128, S], F32R, tag="xt", name=f"xt1_{d}")
                        nc.sync.dma_start(out=t_, in_=xT[1, 128 * d : 128 * (d + 1), :])
                        xt_next.append(t_)

                # ---- normalize (divide by denom row) and ship to send buffer ----
                for i in range(4):
                    rec = small.tile([1, 512], F32R, tag="rec")
                    with nc.allow_low_precision(reason="fp32r softmax recip feeds fp32r matmul"):
                        nc.vector.reciprocal(rec, o_ps[i][DK : DK + 1, :])
                    ps_bc = psc.tile([DK, 512], F32, tag="sc")
                    nc.tensor.matmul(ps_bc, lhsT=ones1, rhs=rec, start=True, stop=True)
                    recb = ot_pool.tile([DK, 512], F32, tag="recb")
                    nc.scalar.copy(recb, ps_bc)
                    ot = ot_pool.tile([DK, 512], F32R, tag="ot")
                    nc.vector.tensor_tensor(
                        out=ot, in0=o_ps[i][0:DK, :], in1=recb, op=ALU.mult
                    )
                    for j in range(2):
                        snd = nc.sync.dma_start(
                            out=send_h[b][
                                16 * (2 * i + j) : 16 * (2 * i + j) + 16, :
                            ].rearrange("r (a c) -> (r a) c", a=4),
                            in_=ot[:, 256 * j : 256 * (j + 1)],
                        )
                    last_send[b] = snd

                nc.gpsimd.collective_compute(
                    "AllToAll",
                    ALU.bypass,
                    replica_groups=[list(range(N_CORES))],
                    ins=[send_h[b][:].opt()],
                    outs=[recv_h[b][:].opt()],
                )

            # prefetch the first FFN weight tiles so they stream during the
            # attention/collective window instead of after it
            w1t_pre, w2t_pre = {}, {}
            for f in range(3):
                w1t_pre[f] = wff_pool.tile([128, 512], F32R, tag="w1t", name=f"w1tp{f}")
                nc.sync.dma_start(
                    out=w1t_pre[f][:].rearrange("p (d c) -> p d c", c=128),
                    in_=w1[:, 128 * f : 128 * (f + 1)].rearrange(
                        "(d p) c -> p d c", p=128
                    ),
                )
                w2t_pre[f] = wff_pool.tile([128, 512], F32R, tag="w2t", name=f"w2tp{f}")
                nc.sync.dma_start(out=w2t_pre[f], in_=w2[128 * f : 128 * (f + 1), :])

            # o_catT tiles per batch-half [128(hk), 256(s_local)]

            def layernorm(dst, src, g_t, be_t):
                st = small.tile([128, 6], F32, tag="st")
                nc.vector.bn_stats(st, src)
                mv = small.tile([128, 2], F32, tag="mv")
                nc.vector.bn_aggr(mv, st)
                rstd = small.tile([128, 1], F32, tag="rstd")
                nc.scalar.activation(
                    out=rstd, in_=mv[:, 1:2], func=AF.Sqrt, bias=eps_t, scale=1.0
                )
                nc.vector.reciprocal(rstd, rstd)
                nmr = small.tile([128, 1], F32, tag="nmr")
                nc.vector.tensor_scalar(
                    out=nmr,
                    in0=mv[:, 0:1],
                    scalar1=rstd,
                    scalar2=-1.0,
                    op0=ALU.mult,
                    op1=ALU.mult,
                )
                tn = tmp_pool.tile([128, D], F32, tag="tn")
                nc.scalar.activation(
                    out=tn, in_=src, func=AF.Identity, bias=nmr, scale=rstd
                )
                tg = tmp_pool.tile([128, D], F32, tag="tg")
                nc.vector.tensor_tensor(out=tg, in0=tn, in1=g_t, op=ALU.mult)
                nc.vector.tensor_tensor(out=dst, in0=tg, in1=be_t, op=ALU.add)

            # ---- per batch-half: o_cat, Wo + residual + LN1, hT, FFN ----
            # The b0 half depends only on the first AllToAll; it is pinned
            # behind b1's last send so it fills the second AllToAll window
            # (weights for the FFN are streamed twice - DMA is idle there).
            from concourse.tile import add_dep_helper

            hT = [h_pool.tile([128, 512], F32R, tag="hT", name=f"hT{d}", bufs=4) for d in range(4)]
            h = [None] * 4
            ff2_ps = [None] * 4

            def half_chain(bb, pin_after):
                oc = []
                for cp in range(4):
                    t_ = tmp_pool.tile(
                        [128, 256], F32R, tag="oc", bufs=8, name=f"oc{bb}_{cp}"
                    )
                    dma = nc.sync.dma_start(
                        out=t_,
                        in_=recv_h[bb][32 * cp : 32 * (cp + 1), :].rearrange(
                            "(j r) (a c) -> (j r a) c", j=2, a=4
                        ),
                    )
                    if pin_after is not None:
                        add_dep_helper(
                            dma.ins, pin_after.ins, sync=True,
                            reason="fill the second AllToAll window",
                        )
                    oc.append(t_)
                for il in range(2):
                    i = 2 * bb + il
                    ps_wo = psc.tile([128, 512], F32, tag="sc", name=f"pswo{i}")
                    for cp in range(4):
                        nc.tensor.matmul(
                            ps_wo,
                            lhsT=oc[cp][:, 128 * il : 128 * (il + 1)],
                            rhs=wo_sb[:, 512 * cp : 512 * (cp + 1)],
                            start=(cp == 0),
                            stop=(cp == 3),
                        )
                    t1 = h_pool.tile([128, D], F32, tag="t1", name=f"t1_{i}")
                    nc.vector.tensor_tensor(out=t1, in0=ps_wo, in1=xsbo[i], op=ALU.add)
                    h_i = h_pool.tile([128, D], F32, tag="h", bufs=4, name=f"h{i}")
                    layernorm(h_i, t1, g1_t, be1_t)
                    h[i] = h_i
                    for d in range(4):
                        ps_t = psc.tile([128, 128], F32, tag="sc", name=f"pst{i}_{d}")
                        nc.tensor.transpose(
                            ps_t, h_i[:, 128 * d : 128 * (d + 1)], ident
                        )
                        nc.vector.tensor_copy(
                            hT[d][:, 128 * i : 128 * (i + 1)], ps_t
                        )

            def ffn_pass(bb):
                chunks = (2 * bb, 2 * bb + 1)
                col = 256 * bb
                for i in chunks:
                    ff2_ps[i] = pacc.tile([128, 512], F32, tag="acc", name=f"ff2ps{i}")
                for f in range(16):
                    if bb == 0 and f in w1t_pre:
                        w1t, w2t = w1t_pre[f], w2t_pre[f]
                    else:
                        w1t = wff_pool.tile(
                            [128, 512], F32R, tag="w1t", name=f"w1t_{bb}_{f}"
                        )
                        nc.sync.dma_start(
                            out=w1t[:].rearrange("p (d c) -> p d c", c=128),
                            in_=w1[:, 128 * f : 128 * (f + 1)].rearrange(
                                "(d p) c -> p d c", p=128
                            ),
                        )
                        w2t = wff_pool.tile(
                            [128, 512], F32R, tag="w2t", name=f"w2t_{bb}_{f}"
                        )
                        nc.sync.dma_start(out=w2t, in_=w2[128 * f : 128 * (f + 1), :])
                    ps1 = psc.tile([128, 256], F32, tag="sc", name=f"ps1_{bb}_{f}")
                    for d in range(4):
                        nc.tensor.matmul(
                            ps1,
                            lhsT=w1t[:, 128 * d : 128 * (d + 1)],
                            rhs=hT[d][:, col : col + 256],
                            start=(d == 0),
                            stop=(d == 3),
                        )
                    f1 = f1_pool.tile([128, 256], F32R, tag="f1", name=f"f1_{bb}_{f}")
                    nc.scalar.activation(
                        out=f1, in_=ps1, func=AF.Relu,
                        bias=b1_sb[:, f : f + 1], scale=1.0,
                    )
                    for il in range(2):
                        nc.tensor.matmul(
                            ff2_ps[chunks[il]],
                            lhsT=f1[:, 128 * il : 128 * (il + 1)],
                            rhs=w2t,
                            start=(f == 0),
                            stop=False,
                        )
                for i in chunks:
                    nc.tensor.matmul(
                        ff2_ps[i], lhsT=ones128, rhs=b2r, start=False, stop=True
                    )

            half_chain(0, last_send[1])
            ffn_pass(0)
            half_chain(1, None)
            ffn_pass(1)

            # ---- epilogue: +b2, +h residual, LN2, store ----
            for i in range(4):
                t2 = tmp_pool.tile([128, D], F32, tag="e2")
                nc.vector.tensor_tensor(out=t2, in0=ff2_ps[i], in1=h[i], op=ALU.add)
                o_sb = tmp_pool.tile([128, D], F32, tag="osb")
                layernorm(o_sb, t2, g2_t, be2_t)
                nc.sync.dma_start(
                    out=out_shard[128 * i : 128 * (i + 1), :], in_=o_sb
                )

    _split_excess_waits(nc)
    return nc


_NC_CACHE = {}


def _get_nc():
    if "nc" not in _NC_CACHE:
        _NC_CACHE["nc"] = build_nc()
    return _NC_CACHE["nc"]


def build_in_maps(inputs):
    x = np.asarray(inputs["x"], np.float32)
    Wq = np.asarray(inputs["Wq"], np.float32)
    bq = np.asarray(inputs["bq"], np.float32)
    Wk = np.asarray(inputs["Wk"], np.float32)
    bk = np.asarray(inputs["bk"], np.float32)
    Wv = np.asarray(inputs["Wv"], np.float32)
    bv = np.asarray(inputs["bv"], np.float32)
    Wo = np.asarray(inputs["Wo"], np.float32)
    bo = np.asarray(inputs["bo"], np.float32)
    ln1_g = np.asarray(inputs["ln1_g"], np.float32)
    ln1_b = np.asarray(inputs["ln1_b"], np.float32)
    W1 = np.asarray(inputs["W1"], np.float32)
    b1 = np.asarray(inputs["b1"], np.float32)
    W2 = np.asarray(inputs["W2"], np.float32)
    b2 = np.asarray(inputs["b2"], np.float32)
    ln2_g = np.asarray(inputs["ln2_g"], np.float32)
    ln2_b = np.asarray(inputs["ln2_b"], np.float32)

    xT = np.ascontiguousarray(x.transpose(0, 2, 1))  # [B, D, S]
    # shard c = 256 tokens of batch 0 + 256 tokens of batch 1
    HSH = NSH // 2
    x_shards = [
        np.concatenate([x[0, HSH * c : HSH * (c + 1)], x[1, HSH * c : HSH * (c + 1)]])
        for c in range(N_CORES)
    ]
    b1c = np.ascontiguousarray(b1.reshape(DF // 128, 128).T)  # [128, 16]

    in_maps = []
    for c in range(N_CORES):
        in_maps.append(
            {
                "xT": xT,
                "xsb": np.ascontiguousarray(x_shards[c] + bo[None, :]),
                "wq": np.ascontiguousarray(Wq[c]),
                "wk": np.ascontiguousarray(Wk[c]),
                "wv": np.ascontiguousarray(Wv[c]),
                "bq": np.ascontiguousarray(bq[c].reshape(DK, 1)),
                "bk": np.ascontiguousarray(bk[c].reshape(DK, 1)),
                "bv": np.ascontiguousarray(bv[c]),
                "wo": Wo,
                "w1": W1,
                "b1c": b1c,
                "w2": W2,
                "b2": b2,
                "g1": ln1_g,
                "be1": ln1_b,
                "g2": ln2_g,
                "be2": ln2_b,
            }
        )

    return in_maps


def kernel(**inputs):
    in_maps = build_in_maps(inputs)
    nc = _get_nc()
    res = run_bass_kernel_spmd(nc, in_maps, core_ids=list(range(N_CORES)))
    shards = [res.results[c]["out_shard"] for c in range(N_CORES)]
    HSH = NSH // 2
    out = np.empty((B, S, D), np.float32)
    for c in range(N_CORES):
        out[0, HSH * c : HSH * (c + 1)] = shards[c][:HSH]
        out[1, HSH * c : HSH * (c + 1)] = shards[c][HSH:]
    return out



# revision 20
# speedup vs baseline: 1.3056x; 1.3056x over previous
"""Trainium2 Bass kernel for nn_EncoderLayer (D=512, H=8, DK=64, DF=2048, B=2, S=2048).

Strategy v2 (8 NeuronCores, batch-split tensor parallel):
  - Core c handles batch b=c//4 and heads (2*(c%4), 2*(c%4)+1). Groups of 4
    cores (one per die pair) cover one batch; the post-attention exchange is
    a 4-rank AllToAll within the group (same-die links, half the bytes).
  - All data bf16 (fp32 accumulation in PSUM); weights are pre-swizzled on
    the host into the exact SBUF layout so every DMA moves contiguous >=1KB
    lines in a handful of large transfers.
  - Attention: q/k kept feature-major (zero-padded to K=128); scores in
    PSUM -> exp on ACT -> AV matmul in token-major orientation
    (lhsT=exp-scores) so the softmax denominator lands as a per-partition
    column: reciprocal + tensor_scalar normalize are cheap, then a PE
    transpose produces the feature-major shard for the exchange.
  - Each head's output is shipped in its own AllToAll; the first one
    overlaps the second head's attention. After the exchange: Wo + LN1 +
    FFN + LN2 on this core's 512 tokens.
"""

import numpy as np

import concourse.bass as bass
import concourse.tile as tile
from concourse import mybir
from concourse.bass_utils import run_bass_kernel_spmd
from concourse.masks import make_identity

F32 = mybir.dt.float32
F32R = mybir.dt.float32r
BF16 = mybir.dt.bfloat16
AF = mybir.ActivationFunctionType
ALU = mybir.AluOpType

B, S, D, H, DK, DF = 2, 2048, 512, 8, 64, 2048
N_CORES = 8
NSH = 512  # tokens per core
EPS = 1e-5
GROUPS = [list(range(N_CORES))]
HSH = 256  # tokens per core per batch (shard = 256 of b0 + 256 of b1)

# ---------------------------------------------------------------------------
# Workaround: this walrus build rejects >1 sem wait on CTRL-type (drain)
# instructions. Split the TileContext tail-drain waits onto dedicated
# single-wait nops; the all-engine barrier right after keeps semantics.


def _split_excess_waits(nc, max_waits=1):
    for fn in nc.m.functions:
        for bb in fn.blocks:
            new_list = []
            for inst in bb.instructions:
                si = inst.sync_info
                waits = list(si.on_wait) if si is not None and si.on_wait else []
                if len(waits) > max_waits:
                    keep = waits[:max_waits]
                    extra = waits[max_waits:]
                    si.on_wait = keep
                    for w in extra:
                        nop = mybir.InstNoOp(name=f"I-waitnop-{nc.next_id()}")
                        nop.engine = inst.engine
                        nop.sync_info = mybir.SyncInfo(on_wait=[w], on_update=[])
                        new_list.append(nop)
                new_list.append(inst)
            bb.instructions = new_list


def _bcast_ap(handle, parts, n):
    """AP reading a 1-D DRAM tensor broadcast across `parts` partitions."""
    a = handle[:]
    return bass.AP(tensor=a.tensor, offset=a.offset, ap=[[0, parts], [1, n]])


def _v():
    import os

    return os.environ.get("KV2_VARIANT", "full")


def build_nc():
    nc = bass.Bass(target_bir_lowering=False)

    # ---- kernel I/O (per core; host pre-swizzles to SBUF layouts) ----
    xt_h = nc.dram_tensor("xt", [D, S], BF16, kind="ExternalInput")
    wqk_h = nc.dram_tensor("wqk", [128, 1024], BF16, kind="ExternalInput")
    wv_h = nc.dram_tensor("wv", [128, 512], BF16, kind="ExternalInput")
    wo_h = nc.dram_tensor("wo", [128, 2048], BF16, kind="ExternalInput")
    w1_h = nc.dram_tensor("w1", [128, 8192], BF16, kind="ExternalInput")
    w2_h = nc.dram_tensor("w2", [128, 8192], BF16, kind="ExternalInput")
    xsb_h = nc.dram_tensor("xsb", [NSH, D], BF16, kind="ExternalInput")
    cst_h = nc.dram_tensor("cst", [128, 18], F32, kind="ExternalInput")
    bv_h = nc.dram_tensor("bv", [128, 128], F32, kind="ExternalInput")
    g1_h = nc.dram_tensor("g1", [D], F32, kind="ExternalInput")
    be1_h = nc.dram_tensor("be1", [D], F32, kind="ExternalInput")
    g2_h = nc.dram_tensor("g2", [D], F32, kind="ExternalInput")
    be2_h = nc.dram_tensor("be2", [D], F32, kind="ExternalInput")
    b2_h = nc.dram_tensor("b2", [D], F32R, kind="ExternalInput")
    out_h = nc.dram_tensor("out_shard", [NSH, D], F32, kind="ExternalOutput")

    with tile.TileContext(nc) as tc:
        with (
            tc.tile_pool(name="consts", bufs=1) as consts,
            tc.tile_pool(name="qk", bufs=1) as qk_pool,
            tc.tile_pool(name="vaug", bufs=16) as v_pool,
            tc.tile_pool(name="et", bufs=5) as et_pool,
            tc.tile_pool(name="otn", bufs=4) as ot_pool,
            tc.tile_pool(name="oc", bufs=4) as oc_pool,
            tc.tile_pool(name="hh", bufs=4) as h_pool,
            tc.tile_pool(name="f1p", bufs=3) as f1_pool,
            tc.tile_pool(name="tmps", bufs=4) as tmp_pool,
            tc.tile_pool(name="small", bufs=4) as small,
            tc.tile_pool(name="psc", bufs=2, space="PSUM") as psc,
            tc.tile_pool(name="pacc", bufs=4, space="PSUM") as pacc,
            tc.tile_pool(name="dram", bufs=1, space="DRAM") as dram,
        ):
            # ---- input streams. sync queue: big loads in priority order;
            # gpsimd queue: small consts.
            wqk_sb = consts.tile([128, 1024], BF16, tag="wqk")
            nc.sync.dma_start(out=wqk_sb, in_=wqk_h[:, :])
            xt = []
            for dd in range(4):
                t_ = consts.tile([128, S], BF16, tag=f"xt{dd}")
                nc.sync.dma_start(out=t_, in_=xt_h[128 * dd : 128 * (dd + 1), :])
                xt.append(t_)
            wv_sb = consts.tile([128, 512], BF16, tag="wv")
            nc.sync.dma_start(out=wv_sb, in_=wv_h[:, :])
            wo_sb = consts.tile([128, 2048], BF16, tag="wo")
            nc.sync.dma_start(out=wo_sb, in_=wo_h[:, :])
            w1_sb = consts.tile([128, 8192], BF16, tag="w1")
            nc.sync.dma_start(out=w1_sb, in_=w1_h[:, :])
            w2_sb = consts.tile([128, 8192], BF16, tag="w2")
            nc.sync.dma_start(out=w2_sb, in_=w2_h[:, :])
            xsbo = []
            for i in range(4):
                t_ = consts.tile([128, D], BF16, tag=f"xsbo{i}")
                nc.sync.dma_start(out=t_, in_=xsb_h[128 * i : 128 * (i + 1), :])
                xsbo.append(t_)

            cst_sb = consts.tile([128, 18], F32, tag="cst")
            nc.gpsimd.dma_start(out=cst_sb, in_=cst_h[:, :])
            bv_sb = consts.tile([128, 128], F32, tag="bv")
            nc.gpsimd.dma_start(out=bv_sb, in_=bv_h[:, :])
            g1_t = consts.tile([128, D], F32, tag="g1_t")
            be1_t = consts.tile([128, D], F32, tag="be1_t")
            g2_t = consts.tile([128, D], F32, tag="g2_t")
            be2_t = consts.tile([128, D], F32, tag="be2_t")
            for t_sb, h_d in ((g1_t, g1_h), (be1_t, be1_h), (g2_t, g2_h), (be2_t, be2_h)):
                nc.gpsimd.dma_start(out=t_sb, in_=_bcast_ap(h_d, 128, D))
            b2r = consts.tile([1, D], F32R, tag="b2r")
            nc.gpsimd.dma_start(out=b2r, in_=b2_h[:].rearrange("(o d) -> o d", o=1))

            ident = consts.tile([128, 128], BF16)
            make_identity(nc, ident)
            eps_t = consts.tile([128, 1], F32)
            nc.vector.memset(eps_t, EPS)
            ones128 = consts.tile([1, 128], F32R)
            nc.vector.memset(ones128[:].bitcast(F32), 1.0)

            # ---- QKV projections for both heads of this core ----
            # qT_u/kT_u: [128, S] bf16, rows 0-63 = head u's projection,
            # rows 64-127 zeroed so score matmuls run with K=128.
            qT = [
                qk_pool.tile([128, S], BF16, tag=f"qT{u}", name=f"qT{u}")
                for u in range(2)
            ]
            kT = [
                qk_pool.tile([128, S], BF16, tag=f"kT{u}", name=f"kT{u}")
                for u in range(2)
            ]
            for t_ in qT + kT:
                nc.gpsimd.memset(t_[64:128, :], 0.0)

            for s4 in range(4):
                for qk, dst, bcol in ((0, qT, 0), (1, kT, 1)):
                    ps = psc.tile([128, 512], F32, tag="sc")
                    for dd in range(4):
                        nc.tensor.matmul(
                            ps,
                            lhsT=wqk_sb[:, 256 * dd + 128 * qk : 256 * dd + 128 * (qk + 1)],
                            rhs=xt[dd][:, 512 * s4 : 512 * (s4 + 1)],
                            start=(dd == 0),
                            stop=(dd == 3),
                        )
                    for u in range(2):
                        nc.vector.tensor_scalar_add(
                            dst[u][0:64, 512 * s4 : 512 * (s4 + 1)],
                            ps[64 * u : 64 * (u + 1), :],
                            cst_sb[64 * u : 64 * (u + 1), bcol : bcol + 1],
                        )

            # v_aug[t]: [128 tokens, 130] = [v_h0 (64) | 1 | v_h1 (64) | 1]
            v_aug = []
            for t in range(16):
                va = v_pool.tile([128, 130], BF16, tag="vaug", name=f"va{t}")
                va_v = va[:].rearrange("p (u c) -> p u c", c=65)
                nc.gpsimd.memset(va_v[:, :, 64:65], 1.0)
                psv = psc.tile([128, 128], F32, tag="sc")
                for dd in range(4):
                    nc.tensor.matmul(
                        psv,
                        lhsT=xt[dd][:, 128 * t : 128 * (t + 1)],
                        rhs=wv_sb[:, 128 * dd : 128 * (dd + 1)],
                        start=(dd == 0),
                        stop=(dd == 3),
                    )
                nc.vector.tensor_tensor(
                    out=va_v[:, :, 0:64],
                    in0=psv[:].rearrange("p (u c) -> p u c", c=64),
                    in1=bv_sb[:].rearrange("p (u c) -> p u c", c=64),
                    op=ALU.add,
                )
                v_aug.append(va)

            # per-unit exchange buffers: 8 blocks of [64 feats, 256 tokens]
            send_h = [dram.tile([512, 256], BF16, name=f"send{u}") for u in range(2)]
            recv_h = [dram.tile([512, 256], BF16, name=f"recv{u}") for u in range(2)]

            # ---- attention per head-unit ----
            for u in range(2):
                # o accumulators: 4 PSUM tiles, each holds 4 s-chunks x 65
                # (64 v-cols + denominator from the ones column).
                o_ps = [
                    pacc.tile([128, 260], F32, tag="acc", name=f"ops{u}_{g}")
                    for g in range(4)
                ]
                et_prev = None

                def emit_av(t, et_half):
                    for half in range(2):
                        et_t = et_half[half]
                        for sl in range(8):
                            s_i = 8 * half + sl
                            nc.tensor.matmul(
                                o_ps[s_i // 4][:, 65 * (s_i % 4) : 65 * (s_i % 4) + 65],
                                lhsT=et_t[:, 128 * sl : 128 * (sl + 1)],
                                rhs=v_aug[t][:, 65 * u : 65 * (u + 1)],
                                start=(t == 0),
                                stop=(t == 15),
                            )

                for t in range(16):
                    et_half = []
                    for half in range(2):
                        ps_sc = psc.tile([128, 1024], F32, tag="sc")
                        for sq in range(2):
                            nc.tensor.matmul(
                                ps_sc[:, 512 * sq : 512 * (sq + 1)],
                                lhsT=kT[u][:, 128 * t : 128 * (t + 1)],
                                rhs=qT[u][:, 1024 * half + 512 * sq : 1024 * half + 512 * (sq + 1)],
                                start=True,
                                stop=True,
                            )
                        et = et_pool.tile([128, 1024], BF16, tag="et")
                        nc.scalar.activation(
                            out=et, in_=ps_sc, func=AF.Exp, bias=0.0, scale=0.125
                        )
                        et_half.append(et)
                    if et_prev is not None:
                        emit_av(t - 1, et_prev)
                    et_prev = et_half
                emit_av(15, et_prev)

                # drain: normalize each s-chunk, transpose to feature-major,
                # collect into oT, ship to the exchange buffer.
                oT = ot_pool.tile([64, S], BF16, tag="oT", name=f"oT{u}")
                for s_i in range(16):
                    g, jj = s_i // 4, s_i % 4
                    rec = small.tile([128, 1], F32, tag="rec")
                    with nc.allow_low_precision(reason="softmax recip"):
                        nc.vector.reciprocal(
                            rec, o_ps[g][:, 65 * jj + 64 : 65 * jj + 65]
                        )
                    o_n = small.tile([128, 64], BF16, tag="o_n")
                    nc.vector.tensor_scalar(
                        out=o_n,
                        in0=o_ps[g][:, 65 * jj : 65 * jj + 64],
                        scalar1=rec,
                        scalar2=None,
                        op0=ALU.mult,
                    )
                    pt = psc.tile([64, 128], BF16, tag="sc", name=f"pt{u}_{s_i}")
                    nc.tensor.transpose(pt, o_n, ident)
                    nc.vector.tensor_copy(oT[:, 128 * s_i : 128 * (s_i + 1)], pt)
                nc.sync.dma_start(
                    out=send_h[u][:].rearrange("(j p) c -> p j c", p=64),
                    in_=oT[:].rearrange("p (j c) -> p j c", j=8),
                )
                nc.gpsimd.collective_compute(
                    "AllToAll",
                    ALU.bypass,
                    replica_groups=GROUPS,
                    ins=[send_h[u][:].opt()],
                    outs=[recv_h[u][:].opt()],
                )

            # ---- token phase: oc assembly, Wo + LN1, FFN, LN2 ----
            # oc tile s (s=0..7: batch s//4, hk-chunk s%4): [128 feats, 256 tok]
            ocT = [
                oc_pool.tile([128, HSH], BF16, tag=f"oc{s}", name=f"oc{s}")
                for s in range(8)
            ]
            for s in range(8):
                for u in range(2):
                    nc.sync.dma_start(
                        out=ocT[s][64 * u : 64 * (u + 1), :],
                        in_=recv_h[u][64 * s : 64 * (s + 1), :],
                    )

            def layernorm(dst, src, g_t, be_t):
                st = small.tile([128, 6], F32, tag="st")
                nc.vector.bn_stats(st, src)
                mv = small.tile([128, 2], F32, tag="mv")
                nc.vector.bn_aggr(mv, st)
                rstd = small.tile([128, 1], F32, tag="rstd")
                nc.scalar.activation(
                    out=rstd, in_=mv[:, 1:2], func=AF.Sqrt, bias=eps_t, scale=1.0
                )
                nc.vector.reciprocal(rstd, rstd)
                nmr = small.tile([128, 1], F32, tag="nmr")
                nc.vector.tensor_scalar(
                    out=nmr,
                    in0=mv[:, 0:1],
                    scalar1=rstd,
                    scalar2=-1.0,
                    op0=ALU.mult,
                    op1=ALU.mult,
                )
                tn = tmp_pool.tile([128, D], F32, tag="tn")
                nc.scalar.activation(
                    out=tn, in_=src, func=AF.Identity, bias=nmr, scale=rstd
                )
                tg = tmp_pool.tile([128, D], F32, tag="tg")
                nc.vector.tensor_tensor(out=tg, in0=tn, in1=g_t, op=ALU.mult)
                nc.vector.tensor_tensor(out=dst, in0=tg, in1=be_t, op=ALU.add)

            h_bf = [None] * 4
            hT = [
                h_pool.tile([128, 512], BF16, tag=f"hT{dd}", name=f"hT{dd}")
                for dd in range(4)
            ]
            for i in range(4):
                # token chunk i: chunks 0,1 = batch 0's 256 tokens; 2,3 = batch 1
                bh, il = i // 2, i % 2
                ps_wo = psc.tile([128, 512], F32, tag="sc", name=f"pswo{i}")
                for r in range(4):
                    nc.tensor.matmul(
                        ps_wo,
                        lhsT=ocT[4 * bh + r][:, 128 * il : 128 * (il + 1)],
                        rhs=wo_sb[:, 512 * r : 512 * (r + 1)],
                        start=(r == 0),
                        stop=(r == 3),
                    )
                t1 = tmp_pool.tile([128, D], F32, tag="t1")
                nc.vector.tensor_tensor(out=t1, in0=ps_wo, in1=xsbo[i], op=ALU.add)
                hb = h_pool.tile([128, D], BF16, tag="h", name=f"h{i}")
                layernorm(hb, t1, g1_t, be1_t)
                h_bf[i] = hb
                for dd in range(4):
                    pt = psc.tile([128, 128], BF16, tag="sc", name=f"ph{i}_{dd}")
                    nc.tensor.transpose(pt, hb[:, 128 * dd : 128 * (dd + 1)], ident)
                    nc.vector.tensor_copy(hT[dd][:, 128 * i : 128 * (i + 1)], pt)

            ff_ps = [
                pacc.tile([128, 512], F32, tag="acc", name=f"ff{i}") for i in range(4)
            ]
            for f in range(16):
                ps1 = psc.tile([128, 512], F32, tag="sc", name=f"ps1_{f}")
                for dd in range(4):
                    nc.tensor.matmul(
                        ps1,
                        lhsT=w1_sb[:, 2048 * dd + 128 * f : 2048 * dd + 128 * (f + 1)],
                        rhs=hT[dd],
                        start=(dd == 0),
                        stop=(dd == 3),
                    )
                f1 = f1_pool.tile([128, 512], BF16, tag="f1", name=f"f1_{f}")
                nc.scalar.activation(
                    out=f1, in_=ps1, func=AF.Relu,
                    bias=cst_sb[:, 2 + f : 3 + f], scale=1.0,
                )
                for i in range(4):
                    nc.tensor.matmul(
                        ff_ps[i],
                        lhsT=f1[:, 128 * i : 128 * (i + 1)],
                        rhs=w2_sb[:, 512 * f : 512 * (f + 1)],
                        start=(f == 0),
                        stop=False,
                    )
            for i in range(4):
                nc.tensor.matmul(
                    ff_ps[i], lhsT=ones128, rhs=b2r, start=False, stop=True
                )

            for i in range(4):
                t2 = tmp_pool.tile([128, D], F32, tag="e2")
                nc.vector.tensor_tensor(out=t2, in0=ff_ps[i], in1=h_bf[i], op=ALU.add)
                o_sb = tmp_pool.tile([128, D], F32, tag="osb")
                layernorm(o_sb, t2, g2_t, be2_t)
                nc.sync.dma_start(out=out_h[128 * i : 128 * (i + 1), :], in_=o_sb)

    _split_excess_waits(nc)
    return nc


_NC_CACHE = {}


def _get_nc():
    if "nc" not in _NC_CACHE:
        _NC_CACHE["nc"] = build_nc()
    return _NC_CACHE["nc"]


def build_in_maps(inputs):
    bf = mybir.dt.np(BF16)
    x = np.asarray(inputs["x"], np.float32)
    Wq = np.asarray(inputs["Wq"], np.float32)
    bq = np.asarray(inputs["bq"], np.float32)
    Wk = np.asarray(inputs["Wk"], np.float32)
    bk = np.asarray(inputs["bk"], np.float32)
    Wv = np.asarray(inputs["Wv"], np.float32)
    bv = np.asarray(inputs["bv"], np.float32)
    Wo = np.asarray(inputs["Wo"], np.float32)
    bo = np.asarray(inputs["bo"], np.float32)
    ln1_g = np.asarray(inputs["ln1_g"], np.float32)
    ln1_b = np.asarray(inputs["ln1_b"], np.float32)
    W1 = np.asarray(inputs["W1"], np.float32)
    b1 = np.asarray(inputs["b1"], np.float32)
    W2 = np.asarray(inputs["W2"], np.float32)
    b2 = np.asarray(inputs["b2"], np.float32)
    ln2_g = np.asarray(inputs["ln2_g"], np.float32)
    ln2_b = np.asarray(inputs["ln2_b"], np.float32)

    def swz(w, p=128):
        # [K, N] -> [p, (K//p)*N] so SBUF partition lines are contiguous
        k, n = w.shape
        return np.ascontiguousarray(
            w.reshape(k // p, p, n).transpose(1, 0, 2).reshape(p, (k // p) * n)
        )

    w1s = swz(W1).astype(bf)
    w2s = swz(W2).astype(bf)
    wos = swz(Wo).astype(bf)
    xts = [np.ascontiguousarray(x[b].T).astype(bf) for b in range(B)]

    in_maps = []
    for c in range(N_CORES):
        b, r = c // 4, c % 4
        h0, h1 = 2 * r, 2 * r + 1
        wqk = swz(
            np.concatenate([Wq[h0], Wq[h1], Wk[h0], Wk[h1]], axis=1)
        ).astype(bf)
        wv2 = swz(np.concatenate([Wv[h0], Wv[h1]], axis=1)).astype(bf)
        cst = np.zeros((128, 18), np.float32)
        cst[0:64, 0] = bq[h0]
        cst[64:128, 0] = bq[h1]
        cst[0:64, 1] = bk[h0]
        cst[64:128, 1] = bk[h1]
        cst[:, 2:18] = b1.reshape(16, 128).T
        bv2 = np.ascontiguousarray(
            np.broadcast_to(
                np.concatenate([bv[h0], bv[h1]])[None, :], (128, 128)
            )
        ).astype(np.float32)
        # token shard of core c: 256 tokens of batch 0 + 256 of batch 1
        xsb = (
            np.concatenate(
                [x[0, HSH * c : HSH * (c + 1)], x[1, HSH * c : HSH * (c + 1)]]
            )
            + bo[None, :]
        ).astype(bf)
        in_maps.append(
            {
                "xt": xts[b],
                "wqk": wqk,
                "wv": wv2,
                "wo": wos,
                "w1": w1s,
                "w2": w2s,
                "xsb": xsb,
                "cst": cst,
                "bv": bv2,
                "g1": ln1_g,
                "be1": ln1_b,
                "g2": ln2_g,
                "be2": ln2_b,
                "b2": b2,
            }
        )
    return in_maps


def kernel(**inputs):
    in_maps = build_in_maps(inputs)
    nc = _get_nc()
    res = run_bass_kernel_spmd(nc, in_maps, core_ids=list(range(N_CORES)))
    out = np.empty((B, S, D), np.float32)
    for c in range(N_CORES):
        sh = res.results[c]["out_shard"]
        out[0, HSH * c : HSH * (c + 1)] = sh[:HSH]
        out[1, HSH * c : HSH * (c + 1)] = sh[HSH:]
    return out


# revision 31
# speedup vs baseline: 1.3492x; 1.0333x over previous
"""Trainium2 Bass kernel for nn_EncoderLayer (D=512, H=8, DK=64, DF=2048, B=2, S=2048).

Strategy v2 (8 NeuronCores, batch-split tensor parallel):
  - Core c handles batch b=c//4 and heads (2*(c%4), 2*(c%4)+1). Groups of 4
    cores (one per die pair) cover one batch; the post-attention exchange is
    a 4-rank AllToAll within the group (same-die links, half the bytes).
  - All data bf16 (fp32 accumulation in PSUM); weights are pre-swizzled on
    the host into the exact SBUF layout so every DMA moves contiguous >=1KB
    lines in a handful of large transfers.
  - Attention: q/k kept feature-major (zero-padded to K=128); scores in
    PSUM -> exp on ACT -> AV matmul in token-major orientation
    (lhsT=exp-scores) so the softmax denominator lands as a per-partition
    column: reciprocal + tensor_scalar normalize are cheap, then a PE
    transpose produces the feature-major shard for the exchange.
  - Each head's output is shipped in its own AllToAll; the first one
    overlaps the second head's attention. After the exchange: Wo + LN1 +
    FFN + LN2 on this core's 512 tokens.
"""

import numpy as np

import concourse.bass as bass
import concourse.tile as tile
from concourse import mybir
from concourse.bass_utils import run_bass_kernel_spmd
from concourse.masks import make_identity

F32 = mybir.dt.float32
F32R = mybir.dt.float32r
BF16 = mybir.dt.bfloat16
AF = mybir.ActivationFunctionType
ALU = mybir.AluOpType

B, S, D, H, DK, DF = 2, 2048, 512, 8, 64, 2048
N_CORES = 8
NSH = 512  # tokens per core
EPS = 1e-5
GROUPS = [list(range(N_CORES))]
HSH = 256  # tokens per core per batch (shard = 256 of b0 + 256 of b1)

# ---------------------------------------------------------------------------
# Workaround: this walrus build rejects >1 sem wait on CTRL-type (drain)
# instructions. Split the TileContext tail-drain waits onto dedicated
# single-wait nops; the all-engine barrier right after keeps semantics.


def _split_excess_waits(nc, max_waits=1):
    for fn in nc.m.functions:
        for bb in fn.blocks:
            new_list = []
            for inst in bb.instructions:
                si = inst.sync_info
                waits = list(si.on_wait) if si is not None and si.on_wait else []
                if len(waits) > max_waits:
                    keep = waits[:max_waits]
                    extra = waits[max_waits:]
                    si.on_wait = keep
                    for w in extra:
                        nop = mybir.InstNoOp(name=f"I-waitnop-{nc.next_id()}")
                        nop.engine = inst.engine
                        nop.sync_info = mybir.SyncInfo(on_wait=[w], on_update=[])
                        new_list.append(nop)
                new_list.append(inst)
            bb.instructions = new_list


def _bcast_ap(handle, parts, n):
    """AP reading a 1-D DRAM tensor broadcast across `parts` partitions."""
    a = handle[:]
    return bass.AP(tensor=a.tensor, offset=a.offset, ap=[[0, parts], [1, n]])


def _v():
    import os

    return os.environ.get("KV2_VARIANT", "full")


def build_nc():
    nc = bass.Bass(target_bir_lowering=False)

    # ---- kernel I/O (per core; host pre-swizzles to SBUF layouts) ----
    xt_h = nc.dram_tensor("xt", [D, S], BF16, kind="ExternalInput")
    wqk_h = nc.dram_tensor("wqk", [128, 1024], BF16, kind="ExternalInput")
    wv_h = nc.dram_tensor("wv", [128, 512], BF16, kind="ExternalInput")
    # Wo split by even/odd head rows so both Wo passes use K=64 at offset 0
    woe_h = nc.dram_tensor("woe", [64, 2048], BF16, kind="ExternalInput")
    woo_h = nc.dram_tensor("woo", [64, 2048], BF16, kind="ExternalInput")
    w1_h = nc.dram_tensor("w1", [128, 8192], BF16, kind="ExternalInput")
    w2_h = nc.dram_tensor("w2", [128, 8192], BF16, kind="ExternalInput")
    xsb_h = nc.dram_tensor("xsb", [NSH, D], BF16, kind="ExternalInput")
    cst_h = nc.dram_tensor("cst", [128, 18], F32, kind="ExternalInput")
    bv_h = nc.dram_tensor("bv", [128, 128], F32, kind="ExternalInput")
    g1_h = nc.dram_tensor("g1", [D], F32, kind="ExternalInput")
    g2_h = nc.dram_tensor("g2", [D], F32, kind="ExternalInput")
    be2_h = nc.dram_tensor("be2", [D], F32, kind="ExternalInput")
    # bb2 = b2 + ln1_b (both added to the pre-LN2 sum via the ones matmul)
    b2_h = nc.dram_tensor("b2", [D], F32R, kind="ExternalInput")
    out_h = nc.dram_tensor("out_shard", [NSH, D], F32, kind="ExternalOutput")

    with tile.TileContext(nc) as tc:
        with (
            tc.tile_pool(name="consts", bufs=1) as consts,
            tc.tile_pool(name="qk", bufs=1) as qk_pool,
            tc.tile_pool(name="vaug", bufs=16) as v_pool,
            tc.tile_pool(name="et", bufs=5) as et_pool,
            tc.tile_pool(name="otn", bufs=2) as ot_pool,
            tc.tile_pool(name="oc", bufs=1) as oc_pool,
            tc.tile_pool(name="hh", bufs=1) as h_pool,
            tc.tile_pool(name="f1p", bufs=3) as f1_pool,
            tc.tile_pool(name="tmps", bufs=2) as tmp_pool,
            tc.tile_pool(name="small", bufs=4) as small,
            tc.tile_pool(name="psc", bufs=2, space="PSUM") as psc,
            tc.tile_pool(name="pacc", bufs=4, space="PSUM") as pacc,
            tc.tile_pool(name="dram", bufs=1, space="DRAM") as dram,
        ):
            # ---- input streams. sync queue: big loads in priority order;
            # gpsimd queue: small consts.
            wqk_sb = consts.tile([128, 1024], BF16, tag="wqk")
            nc.sync.dma_start(out=wqk_sb, in_=wqk_h[:, :])
            xt = []
            for dd in range(4):
                t_ = consts.tile([128, S], BF16, tag=f"xt{dd}")
                nc.sync.dma_start(out=t_, in_=xt_h[128 * dd : 128 * (dd + 1), :])
                xt.append(t_)
            wv_sb = consts.tile([128, 512], BF16, tag="wv")
            nc.sync.dma_start(out=wv_sb, in_=wv_h[:, :])
            woe_sb = consts.tile([64, 2048], BF16, tag="woe")
            nc.sync.dma_start(out=woe_sb, in_=woe_h[:, :])
            woo_sb = consts.tile([64, 2048], BF16, tag="woo")
            nc.sync.dma_start(out=woo_sb, in_=woo_h[:, :])
            w1_sb = consts.tile([128, 8192], BF16, tag="w1")
            nc.sync.dma_start(out=w1_sb, in_=w1_h[:, :])
            w2_sb = consts.tile([128, 8192], BF16, tag="w2")
            nc.sync.dma_start(out=w2_sb, in_=w2_h[:, :])
            xsbo = []
            for i in range(4):
                t_ = consts.tile([128, D], BF16, tag=f"xsbo{i}")
                nc.sync.dma_start(out=t_, in_=xsb_h[128 * i : 128 * (i + 1), :])
                xsbo.append(t_)

            cst_sb = consts.tile([128, 18], F32, tag="cst")
            nc.gpsimd.dma_start(out=cst_sb, in_=cst_h[:, :])
            bv_sb = consts.tile([128, 128], F32, tag="bv")
            nc.gpsimd.dma_start(out=bv_sb, in_=bv_h[:, :])
            g1_t = consts.tile([128, D], F32, tag="g1_t")
            g2_t = consts.tile([128, D], F32, tag="g2_t")
            be2_t = consts.tile([128, D], F32, tag="be2_t")
            for t_sb, h_d in ((g1_t, g1_h), (g2_t, g2_h), (be2_t, be2_h)):
                nc.gpsimd.dma_start(out=t_sb, in_=_bcast_ap(h_d, 128, D))
            b2r = consts.tile([1, D], F32R, tag="b2r")
            nc.gpsimd.dma_start(out=b2r, in_=b2_h[:].rearrange("(o d) -> o d", o=1))

            ident = consts.tile([128, 128], BF16)
            make_identity(nc, ident)
            eps_t = consts.tile([128, 1], F32)
            nc.vector.memset(eps_t, EPS)
            ones128 = consts.tile([1, 128], F32R)
            nc.vector.memset(ones128[:].bitcast(F32), 1.0)

            # ---- QKV projections for both heads of this core ----
            # qT_u/kT_u: [128, S] bf16, rows 0-63 = head u's projection,
            # rows 64-127 zeroed so score matmuls run with K=128.
            qT = [
                qk_pool.tile([128, S], BF16, tag=f"qT{u}", name=f"qT{u}")
                for u in range(2)
            ]
            kT = [
                qk_pool.tile([128, S], BF16, tag=f"kT{u}", name=f"kT{u}")
                for u in range(2)
            ]
            for t_ in qT + kT:
                nc.gpsimd.memset(t_[64:128, :], 0.0)

            for s4 in range(4):
                for qk, dst, bcol in ((0, qT, 0), (1, kT, 1)):
                    ps = psc.tile([128, 512], F32, tag="sc")
                    for dd in range(4):
                        nc.tensor.matmul(
                            ps,
                            lhsT=wqk_sb[:, 256 * dd + 128 * qk : 256 * dd + 128 * (qk + 1)],
                            rhs=xt[dd][:, 512 * s4 : 512 * (s4 + 1)],
                            start=(dd == 0),
                            stop=(dd == 3),
                        )
                    for u in range(2):
                        nc.vector.tensor_scalar_add(
                            dst[u][0:64, 512 * s4 : 512 * (s4 + 1)],
                            ps[64 * u : 64 * (u + 1), :],
                            cst_sb[64 * u : 64 * (u + 1), bcol : bcol + 1],
                        )

            # v_aug[t]: [128 tokens, 130] = [v_h0 (64) | 1 | v_h1 (64) | 1]
            v_aug = []
            for t in range(16):
                va = v_pool.tile([128, 130], BF16, tag="vaug", name=f"va{t}")
                va_v = va[:].rearrange("p (u c) -> p u c", c=65)
                nc.gpsimd.memset(va_v[:, :, 64:65], 1.0)
                psv = psc.tile([128, 128], F32, tag="sc")
                for dd in range(4):
                    nc.tensor.matmul(
                        psv,
                        lhsT=xt[dd][:, 128 * t : 128 * (t + 1)],
                        rhs=wv_sb[:, 128 * dd : 128 * (dd + 1)],
                        start=(dd == 0),
                        stop=(dd == 3),
                    )
                nc.vector.tensor_tensor(
                    out=va_v[:, :, 0:64],
                    in0=psv[:].rearrange("p (u c) -> p u c", c=64),
                    in1=bv_sb[:].rearrange("p (u c) -> p u c", c=64),
                    op=ALU.add,
                )
                v_aug.append(va)

            # per-unit exchange buffers: 8 blocks of [64 feats, 256 tokens]
            send_h = [dram.tile([512, 256], BF16, name=f"send{u}") for u in range(2)]
            recv_h = [dram.tile([512, 256], BF16, name=f"recv{u}") for u in range(2)]

            # ---- attention per head-unit ----
            for u in range(2):
                # o accumulators: 4 PSUM tiles, each holds 4 s-chunks x 65
                # (64 v-cols + denominator from the ones column).
                o_ps = [
                    pacc.tile([128, 260], F32, tag="acc", name=f"ops{u}_{g}")
                    for g in range(4)
                ]
                et_prev = None

                def emit_av(t, et_half):
                    for half in range(2):
                        et_t = et_half[half]
                        for sl in range(8):
                            s_i = 8 * half + sl
                            nc.tensor.matmul(
                                o_ps[s_i // 4][:, 65 * (s_i % 4) : 65 * (s_i % 4) + 65],
                                lhsT=et_t[:, 128 * sl : 128 * (sl + 1)],
                                rhs=v_aug[t][:, 65 * u : 65 * (u + 1)],
                                start=(t == 0),
                                stop=(t == 15),
                            )

                for t in range(16):
                    et_half = []
                    for half in range(2):
                        ps_sc = psc.tile([128, 1024], F32, tag="sc")
                        for sq in range(2):
                            nc.tensor.matmul(
                                ps_sc[:, 512 * sq : 512 * (sq + 1)],
                                lhsT=kT[u][:, 128 * t : 128 * (t + 1)],
                                rhs=qT[u][:, 1024 * half + 512 * sq : 1024 * half + 512 * (sq + 1)],
                                start=True,
                                stop=True,
                            )
                        et = et_pool.tile([128, 1024], BF16, tag="et")
                        nc.scalar.activation(
                            out=et, in_=ps_sc, func=AF.Exp, bias=0.0, scale=0.125
                        )
                        et_half.append(et)
                    if et_prev is not None:
                        emit_av(t - 1, et_prev)
                    et_prev = et_half
                emit_av(15, et_prev)

                # drain: normalize each s-chunk, transpose to feature-major,
                # collect into oT, ship to the exchange buffer.
                oT = ot_pool.tile([64, S], BF16, tag="oT", name=f"oT{u}")
                for s_i in range(16):
                    g, jj = s_i // 4, s_i % 4
                    rec = small.tile([128, 1], F32, tag="rec")
                    with nc.allow_low_precision(reason="softmax recip"):
                        nc.vector.reciprocal(
                            rec, o_ps[g][:, 65 * jj + 64 : 65 * jj + 65]
                        )
                    o_n = small.tile([128, 64], BF16, tag="o_n")
                    nc.vector.tensor_scalar(
                        out=o_n,
                        in0=o_ps[g][:, 65 * jj : 65 * jj + 64],
                        scalar1=rec,
                        scalar2=None,
                        op0=ALU.mult,
                    )
                    pt = psc.tile([64, 128], BF16, tag="sc", name=f"pt{u}_{s_i}")
                    nc.tensor.transpose(pt, o_n, ident)
                    nc.vector.tensor_copy(oT[:, 128 * s_i : 128 * (s_i + 1)], pt)
                nc.sync.dma_start(
                    out=send_h[u][:].rearrange("(j p) c -> p j c", p=64),
                    in_=oT[:].rearrange("p (j c) -> p j c", j=8),
                )
                nc.gpsimd.collective_compute(
                    "AllToAll",
                    ALU.bypass,
                    replica_groups=GROUPS,
                    ins=[send_h[u][:].opt()],
                    outs=[recv_h[u][:].opt()],
                )
                if u == 0:
                    # unit A's blocks land during unit B's attention; load now
                    ocA = [
                        oc_pool.tile([64, HSH], BF16, tag=f"ocA{s}", name=f"ocA{s}")
                        for s in range(8)
                    ]
                    for s in range(8):
                        nc.sync.dma_start(
                            out=ocA[s], in_=recv_h[0][64 * s : 64 * (s + 1), :]
                        )

            # ---- token phase ----
            # Wo in two K=64 passes: pass 1 (even heads, from the first
            # exchange) fills the second AllToAll's latency window; pass 2
            # (odd heads) runs once the second exchange lands.
            ps_wo = [
                pacc.tile([128, 512], F32, tag="acc", name=f"pswo{i}")
                for i in range(4)
            ]
            for i in range(4):
                bh, il = i // 2, i % 2
                for r in range(4):
                    nc.tensor.matmul(
                        ps_wo[i],
                        lhsT=ocA[4 * bh + r][:, 128 * il : 128 * (il + 1)],
                        rhs=woe_sb[:, 512 * r : 512 * (r + 1)],
                        start=(r == 0),
                        stop=False,
                    )
            ocB = [
                oc_pool.tile([64, HSH], BF16, tag=f"ocB{s}", name=f"ocB{s}")
                for s in range(8)
            ]
            for s in range(8):
                eng = nc.sync if s % 2 == 0 else nc.scalar
                eng.dma_start(out=ocB[s], in_=recv_h[1][64 * s : 64 * (s + 1), :])

            def ln_core(dst, src):
                # (x - mu) * rsqrt(var + eps); gamma/beta folded elsewhere
                st = small.tile([128, 6], F32, tag="st")
                nc.vector.bn_stats(st, src)
                mv = small.tile([128, 2], F32, tag="mv")
                nc.vector.bn_aggr(mv, st)
                rstd = small.tile([128, 1], F32, tag="rstd")
                nc.scalar.activation(
                    out=rstd, in_=mv[:, 1:2], func=AF.Sqrt, bias=eps_t, scale=1.0
                )
                nc.vector.reciprocal(rstd, rstd)
                nmr = small.tile([128, 1], F32, tag="nmr")
                nc.vector.tensor_scalar(
                    out=nmr,
                    in0=mv[:, 0:1],
                    scalar1=rstd,
                    scalar2=-1.0,
                    op0=ALU.mult,
                    op1=ALU.mult,
                )
                nc.scalar.activation(
                    out=dst, in_=src, func=AF.Identity, bias=nmr, scale=rstd
                )
                return rstd, nmr

            h_bf = [None] * 4
            hT = [
                h_pool.tile([128, 512], BF16, tag=f"hT{dd}", name=f"hT{dd}")
                for dd in range(4)
            ]
            for i in range(4):
                # token chunk i: chunks 0,1 = batch 0's 256 tokens; 2,3 = batch 1
                bh, il = i // 2, i % 2
                for r in range(4):
                    nc.tensor.matmul(
                        ps_wo[i],
                        lhsT=ocB[4 * bh + r][:, 128 * il : 128 * (il + 1)],
                        rhs=woo_sb[:, 512 * r : 512 * (r + 1)],
                        start=False,
                        stop=(r == 3),
                    )
                t1 = tmp_pool.tile([128, D], F32, tag="t1")
                nc.vector.tensor_tensor(out=t1, in0=ps_wo[i], in1=xsbo[i], op=ALU.add)
                hb = h_pool.tile([128, D], BF16, tag=f"h{i}", name=f"h{i}")
                ln_core(hb, t1)
                h_bf[i] = hb
                for dd in range(4):
                    pt = pacc.tile([128, 128], BF16, tag="acc", name=f"ph{i}_{dd}")
                    nc.tensor.transpose(pt, hb[:, 128 * dd : 128 * (dd + 1)], ident)
                    nc.vector.tensor_copy(hT[dd][:, 128 * i : 128 * (i + 1)], pt)

            ff_ps = [
                pacc.tile([128, 512], F32, tag="acc", name=f"ff{i}") for i in range(4)
            ]
            for f in range(16):
                ps1 = psc.tile([128, 512], F32, tag="sc", name=f"ps1_{f}")
                for dd in range(4):
                    nc.tensor.matmul(
                        ps1,
                        lhsT=w1_sb[:, 2048 * dd + 128 * f : 2048 * dd + 128 * (f + 1)],
                        rhs=hT[dd],
                        start=(dd == 0),
                        stop=(dd == 3),
                    )
                f1 = f1_pool.tile([128, 512], BF16, tag="f1", name=f"f1_{f}")
                nc.scalar.activation(
                    out=f1, in_=ps1, func=AF.Relu,
                    bias=cst_sb[:, 2 + f : 3 + f], scale=1.0,
                )
                for i in range(4):
                    nc.tensor.matmul(
                        ff_ps[i],
                        lhsT=f1[:, 128 * i : 128 * (i + 1)],
                        rhs=w2_sb[:, 512 * f : 512 * (f + 1)],
                        start=(f == 0),
                        stop=False,
                    )
            for i in range(4):
                nc.tensor.matmul(
                    ff_ps[i], lhsT=ones128, rhs=b2r, start=False, stop=True
                )

            for i in range(4):
                # residual 2: ff (+ b2 + ln1_b already accumulated) + h_core*g1
                hg = tmp_pool.tile([128, D], F32, tag="hg")
                nc.vector.tensor_tensor(out=hg, in0=h_bf[i], in1=g1_t, op=ALU.mult)
                t2 = tmp_pool.tile([128, D], F32, tag="e2")
                nc.vector.tensor_tensor(out=t2, in0=ff_ps[i], in1=hg, op=ALU.add)
                tn = tmp_pool.tile([128, D], F32, tag="tn")
                ln_core(tn, t2)
                tg = tmp_pool.tile([128, D], F32, tag="tg")
                nc.vector.tensor_tensor(out=tg, in0=tn, in1=g2_t, op=ALU.mult)
                o_sb = tmp_pool.tile([128, D], F32, tag="osb")
                nc.vector.tensor_tensor(out=o_sb, in0=tg, in1=be2_t, op=ALU.add)
                nc.sync.dma_start(out=out_h[128 * i : 128 * (i + 1), :], in_=o_sb)

    _split_excess_waits(nc)
    return nc


_NC_CACHE = {}


def _get_nc():
    if "nc" not in _NC_CACHE:
        _NC_CACHE["nc"] = build_nc()
    return _NC_CACHE["nc"]


def build_in_maps(inputs):
    bf = mybir.dt.np(BF16)
    x = np.asarray(inputs["x"], np.float32)
    Wq = np.asarray(inputs["Wq"], np.float32)
    bq = np.asarray(inputs["bq"], np.float32)
    Wk = np.asarray(inputs["Wk"], np.float32)
    bk = np.asarray(inputs["bk"], np.float32)
    Wv = np.asarray(inputs["Wv"], np.float32)
    bv = np.asarray(inputs["bv"], np.float32)
    Wo = np.asarray(inputs["Wo"], np.float32)
    bo = np.asarray(inputs["bo"], np.float32)
    ln1_g = np.asarray(inputs["ln1_g"], np.float32)
    ln1_b = np.asarray(inputs["ln1_b"], np.float32)
    W1 = np.asarray(inputs["W1"], np.float32)
    b1 = np.asarray(inputs["b1"], np.float32)
    W2 = np.asarray(inputs["W2"], np.float32)
    b2 = np.asarray(inputs["b2"], np.float32)
    ln2_g = np.asarray(inputs["ln2_g"], np.float32)
    ln2_b = np.asarray(inputs["ln2_b"], np.float32)

    def swz(w, p=128):
        # [K, N] -> [p, (K//p)*N] so SBUF partition lines are contiguous
        k, n = w.shape
        return np.ascontiguousarray(
            w.reshape(k // p, p, n).transpose(1, 0, 2).reshape(p, (k // p) * n)
        )

    # fold LN1 gamma into W1 rows and LN1 beta into the FFN bias / residual:
    #   relu((h*g1 + be1) @ W1 + b1) = relu(h @ (g1*W1) + (be1 @ W1 + b1))
    #   pre-LN2 sum gets + be1 via bb2 = b2 + be1 (ones-matmul path)
    w1s = swz(W1 * ln1_g[:, None]).astype(bf)
    b1f = b1 + ln1_b @ W1
    bb2 = (b2 + ln1_b).astype(np.float32)
    w2s = swz(W2).astype(bf)
    wo4 = Wo.reshape(4, 2, 64, D)
    woe = np.ascontiguousarray(
        wo4[:, 0].transpose(1, 0, 2).reshape(64, 4 * D)
    ).astype(bf)
    woo = np.ascontiguousarray(
        wo4[:, 1].transpose(1, 0, 2).reshape(64, 4 * D)
    ).astype(bf)
    xts = [np.ascontiguousarray(x[b].T).astype(bf) for b in range(B)]

    in_maps = []
    for c in range(N_CORES):
        b, r = c // 4, c % 4
        h0, h1 = 2 * r, 2 * r + 1
        wqk = swz(
            np.concatenate([Wq[h0], Wq[h1], Wk[h0], Wk[h1]], axis=1)
        ).astype(bf)
        wv2 = swz(np.concatenate([Wv[h0], Wv[h1]], axis=1)).astype(bf)
        cst = np.zeros((128, 18), np.float32)
        cst[0:64, 0] = bq[h0]
        cst[64:128, 0] = bq[h1]
        cst[0:64, 1] = bk[h0]
        cst[64:128, 1] = bk[h1]
        cst[:, 2:18] = b1f.reshape(16, 128).T
        bv2 = np.ascontiguousarray(
            np.broadcast_to(
                np.concatenate([bv[h0], bv[h1]])[None, :], (128, 128)
            )
        ).astype(np.float32)
        # token shard of core c: 256 tokens of batch 0 + 256 of batch 1
        xsb = (
            np.concatenate(
                [x[0, HSH * c : HSH * (c + 1)], x[1, HSH * c : HSH * (c + 1)]]
            )
            + bo[None, :]
        ).astype(bf)
        in_maps.append(
            {
                "xt": xts[b],
                "wqk": wqk,
                "wv": wv2,
                "woe": woe,
                "woo": woo,
                "w1": w1s,
                "w2": w2s,
                "xsb": xsb,
                "cst": cst,
                "bv": bv2,
                "g1": ln1_g,
                "g2": ln2_g,
                "be2": ln2_b,
                "b2": bb2,
            }
        )
    return in_maps


def kernel(**inputs):
    in_maps = build_in_maps(inputs)
    nc = _get_nc()
    res = run_bass_kernel_spmd(nc, in_maps, core_ids=list(range(N_CORES)))
    out = np.empty((B, S, D), np.float32)
    for c in range(N_CORES):
        sh = res.results[c]["out_shard"]
        out[0, HSH * c : HSH * (c + 1)] = sh[:HSH]
        out[1, HSH * c : HSH * (c + 1)] = sh[HSH:]
    return out


# revision 35
# speedup vs baseline: 1.4341x; 1.0629x over previous
"""Trainium2 Bass kernel for nn_EncoderLayer (D=512, H=8, DK=64, DF=2048, B=2, S=2048).

Strategy v2 (8 NeuronCores, batch-split tensor parallel):
  - Core c handles batch b=c//4 and heads (2*(c%4), 2*(c%4)+1). Groups of 4
    cores (one per die pair) cover one batch; the post-attention exchange is
    a 4-rank AllToAll within the group (same-die links, half the bytes).
  - All data bf16 (fp32 accumulation in PSUM); weights are pre-swizzled on
    the host into the exact SBUF layout so every DMA moves contiguous >=1KB
    lines in a handful of large transfers.
  - Attention: q/k kept feature-major (zero-padded to K=128); scores in
    PSUM -> exp on ACT -> AV matmul in token-major orientation
    (lhsT=exp-scores) so the softmax denominator lands as a per-partition
    column: reciprocal + tensor_scalar normalize are cheap, then a PE
    transpose produces the feature-major shard for the exchange.
  - Each head's output is shipped in its own AllToAll; the first one
    overlaps the second head's attention. After the exchange: Wo + LN1 +
    FFN + LN2 on this core's 512 tokens.
"""

import numpy as np

import concourse.bass as bass
import concourse.tile as tile
from concourse import mybir
from concourse.bass_utils import run_bass_kernel_spmd
from concourse.masks import make_identity

F32 = mybir.dt.float32
F32R = mybir.dt.float32r
BF16 = mybir.dt.bfloat16
AF = mybir.ActivationFunctionType
ALU = mybir.AluOpType

B, S, D, H, DK, DF = 2, 2048, 512, 8, 64, 2048
N_CORES = 8
NSH = 512  # tokens per core
EPS = 1e-5
GROUPS = [list(range(N_CORES))]
HSH = 256  # tokens per core per batch (shard = 256 of b0 + 256 of b1)

# ---------------------------------------------------------------------------
# Workaround: this walrus build rejects >1 sem wait on CTRL-type (drain)
# instructions. Split the TileContext tail-drain waits onto dedicated
# single-wait nops; the all-engine barrier right after keeps semantics.


def _split_excess_waits(nc, max_waits=1):
    for fn in nc.m.functions:
        for bb in fn.blocks:
            new_list = []
            for inst in bb.instructions:
                si = inst.sync_info
                waits = list(si.on_wait) if si is not None and si.on_wait else []
                if len(waits) > max_waits:
                    keep = waits[:max_waits]
                    extra = waits[max_waits:]
                    si.on_wait = keep
                    for w in extra:
                        nop = mybir.InstNoOp(name=f"I-waitnop-{nc.next_id()}")
                        nop.engine = inst.engine
                        nop.sync_info = mybir.SyncInfo(on_wait=[w], on_update=[])
                        new_list.append(nop)
                new_list.append(inst)
            bb.instructions = new_list


def _bcast_ap(handle, parts, n):
    """AP reading a 1-D DRAM tensor broadcast across `parts` partitions."""
    a = handle[:]
    return bass.AP(tensor=a.tensor, offset=a.offset, ap=[[0, parts], [1, n]])


def _v():
    import os

    return os.environ.get("KV2_VARIANT", "full")


def build_nc():
    nc = bass.Bass(target_bir_lowering=False)

    # ---- kernel I/O (per core; host pre-swizzles to SBUF layouts) ----
    xt_h = nc.dram_tensor("xt", [D, S], BF16, kind="ExternalInput")
    wqk_h = nc.dram_tensor("wqk", [128, 1024], BF16, kind="ExternalInput")
    wv_h = nc.dram_tensor("wv", [128, 512], BF16, kind="ExternalInput")
    # Wo split by even/odd head rows so both Wo passes use K=64 at offset 0
    woe_h = nc.dram_tensor("woe", [64, 2048], BF16, kind="ExternalInput")
    woo_h = nc.dram_tensor("woo", [64, 2048], BF16, kind="ExternalInput")
    w1_h = nc.dram_tensor("w1", [128, 8192], BF16, kind="ExternalInput")
    w2_h = nc.dram_tensor("w2", [128, 8192], BF16, kind="ExternalInput")
    xsb_h = nc.dram_tensor("xsb", [NSH, D], BF16, kind="ExternalInput")
    cst_h = nc.dram_tensor("cst", [128, 18], F32, kind="ExternalInput")
    bv_h = nc.dram_tensor("bv", [128, 128], F32, kind="ExternalInput")
    g1_h = nc.dram_tensor("g1", [D], F32, kind="ExternalInput")
    g2_h = nc.dram_tensor("g2", [D], F32, kind="ExternalInput")
    be2_h = nc.dram_tensor("be2", [D], F32, kind="ExternalInput")
    # bb2 = b2 + ln1_b (both added to the pre-LN2 sum via the ones matmul)
    b2_h = nc.dram_tensor("b2", [D], F32R, kind="ExternalInput")
    out_h = nc.dram_tensor("out_shard", [NSH, D], F32, kind="ExternalOutput")

    with tile.TileContext(nc) as tc:
        with (
            tc.tile_pool(name="consts", bufs=1) as consts,
            tc.tile_pool(name="qk", bufs=1) as qk_pool,
            tc.tile_pool(name="vaug", bufs=16) as v_pool,
            tc.tile_pool(name="et", bufs=5) as et_pool,
            tc.tile_pool(name="otn", bufs=2) as ot_pool,
            tc.tile_pool(name="oc", bufs=1) as oc_pool,
            tc.tile_pool(name="hh", bufs=1) as h_pool,
            tc.tile_pool(name="f1p", bufs=3) as f1_pool,
            tc.tile_pool(name="tmps", bufs=2) as tmp_pool,
            tc.tile_pool(name="small", bufs=4) as small,
            tc.tile_pool(name="psc", bufs=2, space="PSUM") as psc,
            tc.tile_pool(name="pacc", bufs=4, space="PSUM") as pacc,
            tc.tile_pool(name="dram", bufs=1, space="DRAM") as dram,
        ):
            # ---- input streams. sync queue: big loads in priority order;
            # gpsimd queue: small consts.
            wqk_sb = consts.tile([128, 1024], BF16, tag="wqk")
            nc.sync.dma_start(out=wqk_sb, in_=wqk_h[:, :])
            xt = []
            for dd in range(4):
                t_ = consts.tile([128, S], BF16, tag=f"xt{dd}")
                nc.sync.dma_start(out=t_, in_=xt_h[128 * dd : 128 * (dd + 1), :])
                xt.append(t_)
            wv_sb = consts.tile([128, 512], BF16, tag="wv")
            nc.sync.dma_start(out=wv_sb, in_=wv_h[:, :])
            woe_sb = consts.tile([64, 2048], BF16, tag="woe")
            nc.sync.dma_start(out=woe_sb, in_=woe_h[:, :])
            woo_sb = consts.tile([64, 2048], BF16, tag="woo")
            nc.sync.dma_start(out=woo_sb, in_=woo_h[:, :])
            w1_sb = consts.tile([128, 8192], BF16, tag="w1")
            nc.sync.dma_start(out=w1_sb, in_=w1_h[:, :])
            w2_sb = consts.tile([128, 8192], BF16, tag="w2")
            nc.sync.dma_start(out=w2_sb, in_=w2_h[:, :])
            xsbo = []
            for i in range(4):
                t_ = consts.tile([128, D], BF16, tag=f"xsbo{i}")
                nc.sync.dma_start(out=t_, in_=xsb_h[128 * i : 128 * (i + 1), :])
                xsbo.append(t_)

            cst_sb = consts.tile([128, 18], F32, tag="cst")
            nc.gpsimd.dma_start(out=cst_sb, in_=cst_h[:, :])
            bv_sb = consts.tile([128, 128], F32, tag="bv")
            nc.gpsimd.dma_start(out=bv_sb, in_=bv_h[:, :])
            g1_t = consts.tile([128, D], F32, tag="g1_t")
            g2_t = consts.tile([128, D], F32, tag="g2_t")
            be2_t = consts.tile([128, D], F32, tag="be2_t")
            for t_sb, h_d in ((g1_t, g1_h), (g2_t, g2_h), (be2_t, be2_h)):
                nc.gpsimd.dma_start(out=t_sb, in_=_bcast_ap(h_d, 128, D))
            b2r = consts.tile([1, D], F32R, tag="b2r")
            nc.gpsimd.dma_start(out=b2r, in_=b2_h[:].rearrange("(o d) -> o d", o=1))

            ident = consts.tile([128, 128], BF16)
            make_identity(nc, ident)
            eps_t = consts.tile([128, 1], F32)
            nc.vector.memset(eps_t, EPS)
            ones128 = consts.tile([1, 128], F32R)
            nc.vector.memset(ones128[:].bitcast(F32), 1.0)
            warm_src = consts.tile([128, 512], BF16, tag="warm")
            nc.vector.memset(warm_src, 0.25)

            def pe_warm(n, name, pin_after=None):
                # dummy matmuls to hold the PE HAM clock-gate open across
                # windows where real matmul work is briefly unavailable
                wp = psc.tile([128, 512], F32, tag="sc", name=f"warm_{name}")
                first = None
                for k in range(n):
                    mm = nc.tensor.matmul(
                        wp,
                        lhsT=warm_src[:, 0:128],
                        rhs=warm_src,
                        start=True,
                        stop=True,
                    )
                    if first is None:
                        first = mm
                if pin_after is not None and first is not None:
                    tile.add_dep_helper(
                        first.ins, pin_after.ins, sync=True,
                        reason="keep PE warm only after the preceding block",
                    )

            pe_warm(20, "boot")

            # ---- QKV projections for both heads of this core ----
            # qT_u/kT_u: [128, S] bf16, rows 0-63 = head u's projection,
            # rows 64-127 zeroed so score matmuls run with K=128.
            qT = [
                qk_pool.tile([128, S], BF16, tag=f"qT{u}", name=f"qT{u}")
                for u in range(2)
            ]
            kT = [
                qk_pool.tile([128, S], BF16, tag=f"kT{u}", name=f"kT{u}")
                for u in range(2)
            ]
            for t_ in qT + kT:
                nc.gpsimd.memset(t_[64:128, :], 0.0)

            for s4 in range(4):
                for qk, dst, bcol in ((0, qT, 0), (1, kT, 1)):
                    ps = psc.tile([128, 512], F32, tag="sc")
                    for dd in range(4):
                        nc.tensor.matmul(
                            ps,
                            lhsT=wqk_sb[:, 256 * dd + 128 * qk : 256 * dd + 128 * (qk + 1)],
                            rhs=xt[dd][:, 512 * s4 : 512 * (s4 + 1)],
                            start=(dd == 0),
                            stop=(dd == 3),
                        )
                    # head 0 copy+bias on DVE, head 1 on ACT — parallel engines
                    nc.vector.tensor_scalar_add(
                        dst[0][0:64, 512 * s4 : 512 * (s4 + 1)],
                        ps[0:64, :],
                        cst_sb[0:64, bcol : bcol + 1],
                    )
                    nc.scalar.activation(
                        out=dst[1][0:64, 512 * s4 : 512 * (s4 + 1)],
                        in_=ps[64:128, :],
                        func=AF.Identity,
                        bias=cst_sb[64:128, bcol : bcol + 1],
                        scale=1.0,
                    )

            # v_aug[t]: [128 tokens, 130] = [v_h0 (64) | 1 | v_h1 (64) | 1]
            v_aug = []
            for t in range(16):
                va = v_pool.tile([128, 130], BF16, tag="vaug", name=f"va{t}")
                va_v = va[:].rearrange("p (u c) -> p u c", c=65)
                nc.gpsimd.memset(va_v[:, :, 64:65], 1.0)
                psv = psc.tile([128, 128], F32, tag="sc")
                for dd in range(4):
                    nc.tensor.matmul(
                        psv,
                        lhsT=xt[dd][:, 128 * t : 128 * (t + 1)],
                        rhs=wv_sb[:, 128 * dd : 128 * (dd + 1)],
                        start=(dd == 0),
                        stop=(dd == 3),
                    )
                nc.vector.tensor_tensor(
                    out=va_v[:, :, 0:64],
                    in0=psv[:].rearrange("p (u c) -> p u c", c=64),
                    in1=bv_sb[:].rearrange("p (u c) -> p u c", c=64),
                    op=ALU.add,
                )
                v_aug.append(va)

            # per-unit exchange buffers: 8 blocks of [64 feats, 256 tokens]
            send_h = [dram.tile([512, 256], BF16, name=f"send{u}") for u in range(2)]
            recv_h = [dram.tile([512, 256], BF16, name=f"recv{u}") for u in range(2)]

            # ---- attention per head-unit ----
            for u in range(2):
                # o accumulators: 4 PSUM tiles, each holds 4 s-chunks x 65
                # (64 v-cols + denominator from the ones column).
                o_ps = [
                    pacc.tile([128, 260], F32, tag="acc", name=f"ops{u}_{g}")
                    for g in range(4)
                ]
                et_prev = None

                def emit_av(t, et_half):
                    for half in range(2):
                        et_t = et_half[half]
                        for sl in range(8):
                            s_i = 8 * half + sl
                            nc.tensor.matmul(
                                o_ps[s_i // 4][:, 65 * (s_i % 4) : 65 * (s_i % 4) + 65],
                                lhsT=et_t[:, 128 * sl : 128 * (sl + 1)],
                                rhs=v_aug[t][:, 65 * u : 65 * (u + 1)],
                                start=(t == 0),
                                stop=(t == 15),
                            )

                for t in range(16):
                    et_half = []
                    for half in range(2):
                        ps_sc = psc.tile([128, 1024], F32, tag="sc")
                        for sq in range(2):
                            nc.tensor.matmul(
                                ps_sc[:, 512 * sq : 512 * (sq + 1)],
                                lhsT=kT[u][:, 128 * t : 128 * (t + 1)],
                                rhs=qT[u][:, 1024 * half + 512 * sq : 1024 * half + 512 * (sq + 1)],
                                start=True,
                                stop=True,
                            )
                        et = et_pool.tile([128, 1024], BF16, tag="et")
                        nc.scalar.activation(
                            out=et, in_=ps_sc, func=AF.Exp, bias=0.0, scale=0.125
                        )
                        et_half.append(et)
                    if et_prev is not None:
                        emit_av(t - 1, et_prev)
                    et_prev = et_half
                emit_av(15, et_prev)

                # drain: normalize each s-chunk, transpose to feature-major,
                # collect into oT, ship to the exchange buffer.
                oT = ot_pool.tile([64, S], BF16, tag="oT", name=f"oT{u}")
                for s_i in range(16):
                    g, jj = s_i // 4, s_i % 4
                    rec = small.tile([128, 1], F32, tag="rec")
                    with nc.allow_low_precision(reason="softmax recip"):
                        nc.vector.reciprocal(
                            rec, o_ps[g][:, 65 * jj + 64 : 65 * jj + 65]
                        )
                    o_n = small.tile([128, 64], BF16, tag="o_n")
                    nc.vector.tensor_scalar(
                        out=o_n,
                        in0=o_ps[g][:, 65 * jj : 65 * jj + 64],
                        scalar1=rec,
                        scalar2=None,
                        op0=ALU.mult,
                    )
                    pt = psc.tile([64, 128], BF16, tag="sc", name=f"pt{u}_{s_i}")
                    nc.tensor.transpose(pt, o_n, ident)
                    nc.vector.tensor_copy(oT[:, 128 * s_i : 128 * (s_i + 1)], pt)
                nc.sync.dma_start(
                    out=send_h[u][:].rearrange("(j p) c -> p j c", p=64),
                    in_=oT[:].rearrange("p (j c) -> p j c", j=8),
                )
                nc.gpsimd.collective_compute(
                    "AllToAll",
                    ALU.bypass,
                    replica_groups=GROUPS,
                    ins=[send_h[u][:].opt()],
                    outs=[recv_h[u][:].opt()],
                )

            # ---- token phase ----
            # unit A's blocks landed during unit B's attention; emitted after
            # unit B's send so they don't block it in the sync-queue FIFO
            ocA = [
                oc_pool.tile([64, HSH], BF16, tag=f"ocA{s}", name=f"ocA{s}")
                for s in range(8)
            ]
            for s in range(8):
                nc.sync.dma_start(
                    out=ocA[s], in_=recv_h[0][64 * s : 64 * (s + 1), :]
                )
            # Wo in two K=64 passes: pass 1 (even heads, from the first
            # exchange) fills the second AllToAll's latency window; pass 2
            # (odd heads) runs once the second exchange lands.
            ps_wo = [
                pacc.tile([128, 512], F32, tag="acc", name=f"pswo{i}")
                for i in range(4)
            ]
            last_p1 = None
            for i in range(4):
                bh, il = i // 2, i % 2
                for r in range(4):
                    last_p1 = nc.tensor.matmul(
                        ps_wo[i],
                        lhsT=ocA[4 * bh + r][:, 128 * il : 128 * (il + 1)],
                        rhs=woe_sb[:, 512 * r : 512 * (r + 1)],
                        start=(r == 0),
                        stop=False,
                    )
            # bridge the second AllToAll's latency so the FFN starts warm
            pe_warm(32, "a2a", pin_after=last_p1)
            ocB = [
                oc_pool.tile([64, HSH], BF16, tag=f"ocB{s}", name=f"ocB{s}")
                for s in range(8)
            ]
            for s in range(8):
                eng = nc.sync if s % 2 == 0 else nc.scalar
                eng.dma_start(out=ocB[s], in_=recv_h[1][64 * s : 64 * (s + 1), :])

            def ln_core(dst, src):
                # (x - mu) * rsqrt(var + eps); gamma/beta folded elsewhere
                st = small.tile([128, 6], F32, tag="st")
                nc.vector.bn_stats(st, src)
                mv = small.tile([128, 2], F32, tag="mv")
                nc.vector.bn_aggr(mv, st)
                rstd = small.tile([128, 1], F32, tag="rstd")
                nc.scalar.activation(
                    out=rstd, in_=mv[:, 1:2], func=AF.Sqrt, bias=eps_t, scale=1.0
                )
                nc.vector.reciprocal(rstd, rstd)
                nmr = small.tile([128, 1], F32, tag="nmr")
                nc.vector.tensor_scalar(
                    out=nmr,
                    in0=mv[:, 0:1],
                    scalar1=rstd,
                    scalar2=-1.0,
                    op0=ALU.mult,
                    op1=ALU.mult,
                )
                nc.scalar.activation(
                    out=dst, in_=src, func=AF.Identity, bias=nmr, scale=rstd
                )
                return rstd, nmr

            h_bf = [None] * 4
            hT = [
                h_pool.tile([128, 512], BF16, tag=f"hT{dd}", name=f"hT{dd}")
                for dd in range(4)
            ]
            for i in range(4):
                # token chunk i: chunks 0,1 = batch 0's 256 tokens; 2,3 = batch 1
                bh, il = i // 2, i % 2
                for r in range(4):
                    nc.tensor.matmul(
                        ps_wo[i],
                        lhsT=ocB[4 * bh + r][:, 128 * il : 128 * (il + 1)],
                        rhs=woo_sb[:, 512 * r : 512 * (r + 1)],
                        start=False,
                        stop=(r == 3),
                    )
                t1 = tmp_pool.tile([128, D], F32, tag="t1")
                nc.vector.tensor_tensor(out=t1, in0=ps_wo[i], in1=xsbo[i], op=ALU.add)
                hb = h_pool.tile([128, D], BF16, tag=f"h{i}", name=f"h{i}")
                ln_core(hb, t1)
                h_bf[i] = hb
                for dd in range(4):
                    pt = pacc.tile([128, 128], BF16, tag="acc", name=f"ph{i}_{dd}")
                    nc.tensor.transpose(pt, hb[:, 128 * dd : 128 * (dd + 1)], ident)
                    nc.vector.tensor_copy(hT[dd][:, 128 * i : 128 * (i + 1)], pt)

            ff_ps = [
                pacc.tile([128, 512], F32, tag="acc", name=f"ff{i}") for i in range(4)
            ]
            for f in range(16):
                ps1 = psc.tile([128, 512], F32, tag="sc", name=f"ps1_{f}")
                for dd in range(4):
                    nc.tensor.matmul(
                        ps1,
                        lhsT=w1_sb[:, 2048 * dd + 128 * f : 2048 * dd + 128 * (f + 1)],
                        rhs=hT[dd],
                        start=(dd == 0),
                        stop=(dd == 3),
                    )
                f1 = f1_pool.tile([128, 512], BF16, tag="f1", name=f"f1_{f}")
                nc.scalar.activation(
                    out=f1, in_=ps1, func=AF.Relu,
                    bias=cst_sb[:, 2 + f : 3 + f], scale=1.0,
                )
                for i in range(4):
                    nc.tensor.matmul(
                        ff_ps[i],
                        lhsT=f1[:, 128 * i : 128 * (i + 1)],
                        rhs=w2_sb[:, 512 * f : 512 * (f + 1)],
                        start=(f == 0),
                        stop=False,
                    )
            for i in range(4):
                nc.tensor.matmul(
                    ff_ps[i], lhsT=ones128, rhs=b2r, start=False, stop=True
                )

            for i in range(4):
                # residual 2: ff (+ b2 + ln1_b already accumulated) + h_core*g1
                hg = tmp_pool.tile([128, D], F32, tag="hg")
                nc.vector.tensor_tensor(out=hg, in0=h_bf[i], in1=g1_t, op=ALU.mult)
                t2 = tmp_pool.tile([128, D], F32, tag="e2")
                nc.vector.tensor_tensor(out=t2, in0=ff_ps[i], in1=hg, op=ALU.add)
                tn = tmp_pool.tile([128, D], F32, tag="tn")
                ln_core(tn, t2)
                tg = tmp_pool.tile([128, D], F32, tag="tg")
                nc.vector.tensor_tensor(out=tg, in0=tn, in1=g2_t, op=ALU.mult)
                o_sb = tmp_pool.tile([128, D], F32, tag="osb")
                nc.vector.tensor_tensor(out=o_sb, in0=tg, in1=be2_t, op=ALU.add)
                nc.sync.dma_start(out=out_h[128 * i : 128 * (i + 1), :], in_=o_sb)

    _split_excess_waits(nc)
    return nc


_NC_CACHE = {}


def _get_nc():
    if "nc" not in _NC_CACHE:
        _NC_CACHE["nc"] = build_nc()
    return _NC_CACHE["nc"]


def build_in_maps(inputs):
    bf = mybir.dt.np(BF16)
    x = np.asarray(inputs["x"], np.float32)
    Wq = np.asarray(inputs["Wq"], np.float32)
    bq = np.asarray(inputs["bq"], np.float32)
    Wk = np.asarray(inputs["Wk"], np.float32)
    bk = np.asarray(inputs["bk"], np.float32)
    Wv = np.asarray(inputs["Wv"], np.float32)
    bv = np.asarray(inputs["bv"], np.float32)
    Wo = np.asarray(inputs["Wo"], np.float32)
    bo = np.asarray(inputs["bo"], np.float32)
    ln1_g = np.asarray(inputs["ln1_g"], np.float32)
    ln1_b = np.asarray(inputs["ln1_b"], np.float32)
    W1 = np.asarray(inputs["W1"], np.float32)
    b1 = np.asarray(inputs["b1"], np.float32)
    W2 = np.asarray(inputs["W2"], np.float32)
    b2 = np.asarray(inputs["b2"], np.float32)
    ln2_g = np.asarray(inputs["ln2_g"], np.float32)
    ln2_b = np.asarray(inputs["ln2_b"], np.float32)

    def swz(w, p=128):
        # [K, N] -> [p, (K//p)*N] so SBUF partition lines are contiguous
        k, n = w.shape
        return np.ascontiguousarray(
            w.reshape(k // p, p, n).transpose(1, 0, 2).reshape(p, (k // p) * n)
        )

    # fold LN1 gamma into W1 rows and LN1 beta into the FFN bias / residual:
    #   relu((h*g1 + be1) @ W1 + b1) = relu(h @ (g1*W1) + (be1 @ W1 + b1))
    #   pre-LN2 sum gets + be1 via bb2 = b2 + be1 (ones-matmul path)
    w1s = swz(W1 * ln1_g[:, None]).astype(bf)
    b1f = b1 + ln1_b @ W1
    bb2 = (b2 + ln1_b).astype(np.float32)
    w2s = swz(W2).astype(bf)
    wo4 = Wo.reshape(4, 2, 64, D)
    woe = np.ascontiguousarray(
        wo4[:, 0].transpose(1, 0, 2).reshape(64, 4 * D)
    ).astype(bf)
    woo = np.ascontiguousarray(
        wo4[:, 1].transpose(1, 0, 2).reshape(64, 4 * D)
    ).astype(bf)
    xts = [np.ascontiguousarray(x[b].T).astype(bf) for b in range(B)]

    in_maps = []
    for c in range(N_CORES):
        b, r = c // 4, c % 4
        h0, h1 = 2 * r, 2 * r + 1
        wqk = swz(
            np.concatenate([Wq[h0], Wq[h1], Wk[h0], Wk[h1]], axis=1)
        ).astype(bf)
        wv2 = swz(np.concatenate([Wv[h0], Wv[h1]], axis=1)).astype(bf)
        cst = np.zeros((128, 18), np.float32)
        cst[0:64, 0] = bq[h0]
        cst[64:128, 0] = bq[h1]
        cst[0:64, 1] = bk[h0]
        cst[64:128, 1] = bk[h1]
        cst[:, 2:18] = b1f.reshape(16, 128).T
        bv2 = np.ascontiguousarray(
            np.broadcast_to(
                np.concatenate([bv[h0], bv[h1]])[None, :], (128, 128)
            )
        ).astype(np.float32)
        # token shard of core c: 256 tokens of batch 0 + 256 of batch 1
        xsb = (
            np.concatenate(
                [x[0, HSH * c : HSH * (c + 1)], x[1, HSH * c : HSH * (c + 1)]]
            )
            + bo[None, :]
        ).astype(bf)
        in_maps.append(
            {
                "xt": xts[b],
                "wqk": wqk,
                "wv": wv2,
                "woe": woe,
                "woo": woo,
                "w1": w1s,
                "w2": w2s,
                "xsb": xsb,
                "cst": cst,
                "bv": bv2,
                "g1": ln1_g,
                "g2": ln2_g,
                "be2": ln2_b,
                "b2": bb2,
            }
        )
    return in_maps


def kernel(**inputs):
    in_maps = build_in_maps(inputs)
    nc = _get_nc()
    res = run_bass_kernel_spmd(nc, in_maps, core_ids=list(range(N_CORES)))
    out = np.empty((B, S, D), np.float32)
    for c in range(N_CORES):
        sh = res.results[c]["out_shard"]
        out[0, HSH * c : HSH * (c + 1)] = sh[:HSH]
        out[1, HSH * c : HSH * (c + 1)] = sh[HSH:]
    return out
